# revision 33
# baseline (speedup 1.0000x reference)
"""Masked dot-product attention on 8 Trainium2 NeuronCores.

Problem: q,k,v [64, 1024, 64] f32, valid_lens [64] int32.
  scores = q @ k^T / 8, mask keys >= valid_len to -1e6, softmax, @ v.

Per core: 8 batches, pure data parallelism, no collectives.  Host prep:
q,k pre-transposed to [D, S] (q also scaled by log2e/8 so device exps
are base-2), v rows past valid_len pre-zeroed with the 0/1 mask as a
65th column (vm) -- the masked softmax denominator falls out of the same
matmul as attn @ v.  Per-batch key tiles truncated to ceil(valid/128);
batches rank-sorted into 8 slots (one per core per slot, same baked
schedule everywhere).

The previous 72us version was exp-bound: 38 [128,1024] Exp ACTIVATEs
x 1.11us on ScalarE were the whole matmul window.  This version:

  - exp runs on TWO engines.  ScalarE keeps exact Exp (scale=ln2,
    bias=-3); ~8 tiles/core instead use a DVE Schraudolph chain:
      u16 = rint(x*1024 + C)          (PSUM f32 -> SBUF u16, ~1.2us)
      u16b = u16 + 512                (int add, 2x mode)
      yb = bitcast_f16(u16b)*2^-0.5   (f16, 2x)
      ex = yb + bitcast_f16(u16)      (f16 add)
    The u16 bit pattern read as f16 is 2^x with the mantissa linearly
    interpolating 2^frac; averaging two half-period phases cuts the
    sawtooth to ~+-0.85% rel.  C folds the f16 exponent bias, the avg2
    pre-halving, the sawtooth centering and the e^-3 exp bias, so
    ScalarE and DVE tiles mix freely within a batch (end-to-end error
    ~1.8e-3).  Only the first op reads PSUM; ops 2-4 are deferred to the
    end of the batch so the score-PSUM recycle never waits on the DVE
    queue's backlog.
  - PE p-state warm-up: the Tensor engine starts at ~1.2GHz and reaches
    2.4GHz only after ~3us of continuous work, and any idle resets it.
    A dummy-matmul burst during the input-DMA dead window plus small
    top-of-slot bursts in the first batches ramp it early and keep it
    ramped; score matmuls then run ~372ns instead of ~630ns.
  - slot order [S3,S4,S5,S7,S6,S2,S1,S0] (ascending rank groups): a
    medium slot first (its exps cover slot 1's DMA), the two largest
    mid-kernel (long continuous PE stretches hold the p-state), the two
    smallest last (short tail).
  - leftover attn@v of a batch is drained AFTER the next batch's score
    emission, never before it, so the burst that unblocks when the last
    exp of a batch lands cannot queue ahead of the scores that feed the
    exp engines.  The drip is sized to drain the WHOLE leftover during
    the next batch's score phase (divisor nxt, not 2*nxt): leaving a
    residual burst at the slot boundary parks 16-48 matmuls in front of
    the following batch's first scores and stalls both exp engines
    ~0.5-1.7us per slot (measured ~2-3us total win from this).
  - epilogue per PSUM half: one [128,4] reciprocal + one broadcast
    (0-stride AP) scalar_tensor_tensor scale for 4 chunks at once, then
    a 128KB half store; the last slot stores in 64KB quarters on both
    DMA queues.
  - pools sized so ex buffers never wrap (no WAR waits; exp waits ride
    on the instructions), inputs prefetched one slot ahead, kT
    dispatched before qT; on the first slot both kT (first 2 key tiles)
    and qT (first 512 cols) are column-split so the first score pair's
    data lands ~1us earlier, and the warm-tile memset runs on the DVE
    so it never delays the gpsimd queue's first input DMA trigger.
"""

import numpy as np

import concourse.bass as bass
import concourse.bacc as bacc
import concourse.tile as tile
from concourse import mybir
from concourse import bass_utils

B, S, D = 64, 1024, 64
NCORES = 8
NB = B // NCORES  # batch slots per core
P = 128
NJT = S // P  # max key tiles per batch
W = D + 1  # v columns + mask column
F32 = mybir.dt.float32
F16 = mybir.dt.float16
U16 = mybir.dt.uint16

LN2 = float(np.log(2.0))
LOG2E = float(np.log2(np.e))
SCH_CORR = 55.0
# u16 = x*1024 + C; C folds the f16 bias (15*1024), the avg2 pre-halving
# (-1024), the sawtooth centering, and the exp bias e^-3 (ScalarE tiles
# use bias=-3, so DVE tiles fold -3*log2e into the exponent domain).
SCH_C = float(15 * 1024 - 1024 - SCH_CORR - 3.0 * LOG2E * 1024)
SQRT_HALF = float(2.0 ** -0.5)

# fraction of each batch's tiles routed to the DVE chain
DVE_FRAC = 0.27

TRACE = False  # set by test harness to capture an NTFF profile
LAST_RESULTS = None  # BassKernelResults stash for the harness

_program_cache = {}


def _av_steps(nc, po_pool, osb_pool, rec_pool, out, s, jt, exs, vm_t,
              last=False):
    """Yield one emission step at a time: 8 attn@v chunk-groups with the
    normalization epilogue emitted per-chunk as each group closes, plus
    half-output DMAs.  The caller interleaves these steps between the NEXT
    batch's score/exp pairs so the PE queue alternates between feeding the
    exp engines (scores) and draining them (attn@v).

    Output accumulators: 8 query-chunks of [128, 65] (cols 0..63 =
    unnormalized out rows, col 64 = denominator); split 4+4 over two PSUM
    banks, groups alternate banks so two can overlap.  As group qc closes,
    its reciprocal+scale run on DVE and the result lands in osb; the
    output store goes out in two 128KB halves (bank A after its last
    chunk, bank B at the end).
    """
    po = [po_pool.tile([P, 4 * W], F32, tag=f"po{h}", name=f"po{h}")
          for h in range(2)]
    order = [0, 4, 1, 5, 2, 6, 3, 7]  # alternate banks
    osb = osb_pool.tile([P, 8 * D], F32, tag="osb", name="osb")
    for qc in order:
        dst = po[qc // 4]
        col = (qc % 4) * W
        for j in range(jt):
            nc.tensor.matmul(
                dst[:, col:col + W],
                lhsT=exs[j][:, qc * P:(qc + 1) * P],
                rhs=vm_t[:, j * W:(j + 1) * W],
                start=(j == 0), stop=(j == jt - 1),
            )
            # fine-grained steps: never queue more than ~4 attn@v matmuls
            # ahead of the next batch's scores, or the exp engines starve
            if j % 4 == 3:
                yield
        if qc in (3, 7):
            # a bank's 4 chunk groups all closed: one [128,4] reciprocal,
            # one broadcast scalar_tensor_tensor scale, half-output DMA
            h = qc // 4
            po3 = po[h].rearrange("p (c w) -> p c w", w=W)
            recp = rec_pool.tile([P, 4], F32, tag=f"rec{h}", name="recp")
            nc.vector.reciprocal(out=recp, in_=po3[:, :, D])
            nc.vector.scalar_tensor_tensor(
                out=osb[:, h * 4 * D:(h + 1) * 4 * D].rearrange(
                    "p (c d) -> p c d", d=D),
                in0=po3[:, :, 0:D], scalar=0.0,
                in1=recp[:, :, None].broadcast_to((P, 4, D)),
                op0=mybir.AluOpType.bypass, op1=mybir.AluOpType.mult)
            if last:
                # tail: quarter stores on both queues so the final
                # transfer is only 64KB
                for qq in range(2):
                    c0 = h * 4 + qq * 2
                    eng = nc.gpsimd if qq == 0 else nc.sync
                    eng.dma_start(
                        out=out[s, c0 * P:(c0 + 2) * P].rearrange(
                            "(c p) d -> p c d", p=P),
                        in_=osb[:, c0 * D:(c0 + 2) * D].rearrange(
                            "p (c d) -> p c d", d=D),
                    )
            else:
                eng = nc.gpsimd if h == 0 else nc.sync
                eng.dma_start(
                    out=out[s, h * 4 * P:(h + 1) * 4 * P].rearrange(
                        "(c p) d -> p c d", p=P),
                    in_=osb[:, h * 4 * D:(h + 1) * 4 * D].rearrange(
                        "p (c d) -> p c d", d=D),
                )
        yield


def _build_program(slots):
    """slots: tuple of (jt, ndve) per batch slot."""
    nc = bacc.Bacc("TRN2", target_bir_lowering=False, debug=False,
                   num_devices=NCORES)
    qT = nc.dram_tensor("qT", [NB, D, S], F16, kind="ExternalInput").ap()
    kT = nc.dram_tensor("kT", [NB, D, S], F16, kind="ExternalInput").ap()
    vm = nc.dram_tensor("vm", [NB, S, W], F16, kind="ExternalInput").ap()
    out = nc.dram_tensor("out", [NB, S, D], F32, kind="ExternalOutput").ap()

    sum_jt = sum(jt for jt, _ in slots)

    with tile.TileContext(nc) as tc:
        with (
            tc.tile_pool(name="singles", bufs=1) as singles,
            tc.tile_pool(name="qk", bufs=3) as qk_pool,
            tc.tile_pool(name="vmp", bufs=4) as vm_pool,
            tc.tile_pool(name="ex", bufs=sum_jt) as ex_pool,
            tc.tile_pool(name="sch", bufs=4) as sch_pool,
            tc.tile_pool(name="osb", bufs=3) as osb_pool,
            tc.tile_pool(name="rec", bufs=2) as rec_pool,
            tc.tile_pool(name="ps_s", bufs=3, space="PSUM") as ps_pool,
            tc.tile_pool(name="ps_o", bufs=1, space="PSUM") as po_pool,
        ):
            # ScalarE tiles: exp(x*ln2 - 3) on x = qk*log2e/8 (qT
            # pre-scaled); -3 bounds the fp16 exp range and cancels
            # between numerator and denominator.
            bias_t = singles.tile([P, 1], F32)
            nc.vector.memset(bias_t, -3.0)

            # PE warm-up: the Tensor engine starts at a low p-state and
            # only reaches full clock after ~3us of continuous work; a
            # burst of dummy matmuls during the otherwise-dead input-DMA
            # window ramps it before the first real score matmul, and
            # small top-of-slot bursts early on keep the ramp alive
            # through the not-yet-pipelined first batches.
            warm = singles.tile([P, 512], F16)
            nc.vector.memset(warm, 1.0)

            def warmup(n):
                wps = ps_pool.tile([P, S], F32, tag="ps", name="wps")
                for _ in range(n):
                    nc.tensor.matmul(wps[:, 0:512], lhsT=warm[:, 0:P],
                                     rhs=warm, start=True, stop=True)

            def emit_input_dmas(s, jt, first=False):
                # q/k replicated into both partition halves (0-stride DMA
                # source) so score matmuls for two key-tiles can run
                # concurrently on PE row-groups (0..63) and (64..127).
                # kT before qT (LDWEIGHTS gates first); on the first slot
                # qT is column-split so the first score pair starts early.
                qT_t = qk_pool.tile([2 * D, S], F16, tag="qT", name="qT_t")
                kT_t = qk_pool.tile([2 * D, S], F16, tag="kT", name="kT_t")
                if first:
                    c0 = min(2 * P, jt * P)
                    nc.sync.dma_start(out=kT_t[0:D, 0:c0],
                                      in_=kT[s, :, 0:c0])
                    nc.gpsimd.dma_start(out=kT_t[D:2 * D, 0:c0],
                                        in_=kT[s, :, 0:c0])
                else:
                    nc.sync.dma_start(out=kT_t[0:D, 0:jt * P],
                                      in_=kT[s, :, 0:jt * P])
                    nc.gpsimd.dma_start(out=kT_t[D:2 * D, 0:jt * P],
                                        in_=kT[s, :, 0:jt * P])
                if first:
                    nc.sync.dma_start(out=qT_t[0:D, 0:512],
                                      in_=qT[s, :, 0:512])
                    nc.gpsimd.dma_start(out=qT_t[D:2 * D, 0:512],
                                        in_=qT[s, :, 0:512])
                    nc.sync.dma_start(out=qT_t[0:D, 512:S],
                                      in_=qT[s, :, 512:S])
                    nc.gpsimd.dma_start(out=qT_t[D:2 * D, 512:S],
                                        in_=qT[s, :, 512:S])
                    if jt * P > c0:
                        nc.sync.dma_start(out=kT_t[0:D, c0:jt * P],
                                          in_=kT[s, :, c0:jt * P])
                        nc.gpsimd.dma_start(out=kT_t[D:2 * D, c0:jt * P],
                                            in_=kT[s, :, c0:jt * P])
                else:
                    nc.sync.dma_start(out=qT_t[0:D, :], in_=qT[s])
                    nc.gpsimd.dma_start(out=qT_t[D:2 * D, :], in_=qT[s])
                # All key tiles of vm in one DMA: [128, jt*65], tile j at
                # columns [j*65, (j+1)*65).
                vm_t = vm_pool.tile([P, NJT * W], F16, tag="vm", name="vm_t")
                nc.sync.dma_start(
                    out=vm_t.rearrange("p (j w) -> p j w", w=W)[:, 0:jt, :],
                    in_=vm[s, 0:jt * P, :].rearrange("(j p) w -> p j w", p=P),
                )
                return qT_t, kT_t, vm_t

            from collections import deque
            pending = deque()  # unfinished attn@v/epilogue generators
            drip = 1
            sch_backlog = []
            warmup(6)
            staged = emit_input_dmas(0, slots[0][0], first=True)
            for s, (jt, ndve) in enumerate(slots):
                qT_t, kT_t, vm_t = staged
                warm_burst = 3 if 1 <= s <= 3 else 0
                if s + 1 < NB:
                    # prefetch the next slot's inputs one slot ahead so
                    # its first score pair never waits on the DMA queue
                    staged = emit_input_dmas(s + 1, slots[s + 1][0])
                # Score matmuls go out in row-group-interleaved pairs --
                # adjacent PE-queue entries on disjoint row groups execute
                # concurrently, so a pair of key tiles costs one tile's time.
                exs = []
                for m in range(0, jt, 2):
                    js = list(range(m, min(m + 2, jt)))
                    pss = [ps_pool.tile([P, S], F32, tag="ps", name="ps")
                           for _ in js]
                    for half in range(2):
                        for r, j in enumerate(js):
                            nc.tensor.matmul(
                                pss[r][:, half * 512:(half + 1) * 512],
                                lhsT=kT_t[r * D:(r + 1) * D,
                                          j * P:(j + 1) * P],
                                rhs=qT_t[r * D:(r + 1) * D,
                                         half * 512:(half + 1) * 512],
                                start=True, stop=True,
                                tile_position=(r * D, 0),
                            )
                    if m == 0 and warm_burst:
                        # p-state hold, placed behind the first pair's
                        # scores instead of in front of them
                        warmup(warm_burst)
                    for r, j in enumerate(js):
                        ex = ex_pool.tile([P, S], F16, tag="ex", name="ex")
                        if j < ndve:
                            # emit only the PSUM-reading op now so the
                            # score buffer frees as fast as an ACTIVATE
                            # would; the SBUF-only ops 2-4 are deferred to
                            # the end of the batch's score phase.
                            ua = sch_pool.tile([P, S], U16, tag="ua",
                                               name="ua")
                            nc.vector.tensor_scalar(
                                out=ua, in0=pss[r], scalar1=1024.0,
                                scalar2=SCH_C, op0=mybir.AluOpType.mult,
                                op1=mybir.AluOpType.add)
                            sch_backlog.append((ua, ex))
                        else:
                            nc.scalar.activation(
                                out=ex, in_=pss[r],
                                func=mybir.ActivationFunctionType.Exp,
                                scale=LN2, bias=bias_t)
                        exs.append(ex)
                        # drain a sliver of the pending attn@v stream
                        # after each exp (keeps the exp engines and PE both
                        # fed), paced to finish just before this batch's
                        # own attn@v
                        for _ in range(drip):
                            if not pending:
                                break
                            if next(pending[0], "done") == "done":
                                pending.popleft()
                for ua, ex in sch_backlog:
                    ub = sch_pool.tile([P, S], U16, tag="ub", name="ub")
                    yb = sch_pool.tile([P, S], F16, tag="yb", name="yb")
                    nc.vector.tensor_scalar(
                        out=ub, in0=ua, scalar1=512, scalar2=None,
                        op0=mybir.AluOpType.add)
                    nc.vector.tensor_scalar(
                        out=yb, in0=ub.bitcast(F16), scalar1=SQRT_HALF,
                        scalar2=None, op0=mybir.AluOpType.mult)
                    nc.vector.tensor_tensor(
                        out=ex, in0=yb, in1=ua.bitcast(F16),
                        op=mybir.AluOpType.add)
                sch_backlog = []
                # drain any leftover attn@v now -- after this batch's
                # scores, so it never blocks them on the PE
                while pending:
                    for _ in pending.popleft():
                        pass
                pending.append(
                    _av_steps(nc, po_pool, osb_pool, rec_pool, out,
                              s, jt, exs, vm_t, last=(s == NB - 1)))
                nsteps = 8 * (jt // 4 + 1)
                nxt = slots[s + 1][0] if s + 1 < NB else jt
                drip = max(1, min(5, -(-nsteps // max(nxt, 1))))
            for gen in pending:
                for _ in gen:
                    pass
    nc.compile()
    return nc


def kernel(q, k, v, valid_lens):
    global LAST_RESULTS
    q = np.array(q, dtype=np.float32, copy=True)
    k = np.asarray(k, dtype=np.float32)
    v = np.asarray(v, dtype=np.float32)
    vl = np.asarray(valid_lens).astype(np.int64)

    # valid_len == 0: reference's softmax over an all-masked row is uniform.
    # Zeroed q gives scores == 0 -> exp == 1 over all (unmasked) keys: same.
    valid_eff = np.where(vl <= 0, S, np.minimum(vl, S))
    q[vl <= 0] = 0.0

    mask = (np.arange(S)[None, :] < valid_eff[:, None]).astype(np.float32)
    # qT carries the 1/8 score scale and log2e: scores become x = s*log2e/8,
    # so exp(s/8) = 2^x for both exp engines.
    qT = np.ascontiguousarray(q.transpose(0, 2, 1) * np.float32(LOG2E / 8))
    qT = qT.astype(np.float16)
    kT = np.ascontiguousarray(k.transpose(0, 2, 1)).astype(np.float16)
    vmh = np.concatenate([v * mask[:, :, None], mask[:, :, None]], axis=2)
    vmh = np.ascontiguousarray(vmh).astype(np.float16)

    # Rank-sort batches; slot s takes one batch of rank group [8s, 8s+8)
    # per core, so the baked per-slot tile count wastes little work.
    # Slot order: ascending, then [.., max, 2nd-max, min]: the pipe fills
    # fast, the big batches' attn@v hides under later exp phases, and the
    # tiny last slot leaves a short tail.
    order = np.argsort(-valid_eff, kind="stable")
    groups = order.reshape(NB, NCORES)[::-1]  # ascending tile counts
    # [S3, S4, S5, S7, S6, S2, S1, S0]: medium slot first (enough exp
    # work to cover the next slot's DMA), the largest batches mid-kernel
    # so the PE gets long continuous stretches (p-state ramp), the two
    # smallest batches last for a short tail.
    perm = [3, 4, 5, NB - 1, NB - 2, 2, 1, 0]
    groups = groups[perm]
    jt_counts = [int(np.ceil(valid_eff[groups[s]].max() / P))
                 for s in range(NB)]
    slots = tuple((jt, int(jt * DVE_FRAC + 0.5) if jt >= 4 else 0)
                  for jt in jt_counts)

    nc = _program_cache.get(slots)
    if nc is None:
        nc = _build_program(slots)
        _program_cache[slots] = nc

    in_maps = []
    for c in range(NCORES):
        bs = groups[:, c]
        in_maps.append({
            "qT": np.ascontiguousarray(qT[bs]),
            "kT": np.ascontiguousarray(kT[bs]),
            "vm": np.ascontiguousarray(vmh[bs]),
        })
    res = bass_utils.run_bass_kernel_spmd(
        nc, in_maps, core_ids=list(range(NCORES)), trace=TRACE,
    )
    LAST_RESULTS = res

    out = np.empty((B, S, D), dtype=np.float32)
    for c in range(NCORES):
        o = res.results[c]["out"]
        for s in range(NB):
            out[groups[s, c]] = o[s]
    return out



# revision 34
# speedup vs baseline: 1.0675x; 1.0675x over previous
"""Masked dot-product attention on 8 Trainium2 NeuronCores.

Problem: q,k,v [64, 1024, 64] f32, valid_lens [64] int32.
  scores = q @ k^T / 8, mask keys >= valid_len to -1e6, softmax, @ v.

Per core: 8 batches, pure data parallelism, no collectives.  Host prep:
q,k pre-transposed to [D, S] (q also scaled by log2e/8 so device exps
are base-2), v rows past valid_len pre-zeroed with the 0/1 mask as a
65th column (vm) -- the masked softmax denominator falls out of the same
matmul as attn @ v.  Per-batch key tiles truncated to ceil(valid/128);
batches rank-sorted into 8 slots (one per core per slot, same baked
schedule everywhere).

The previous 72us version was exp-bound: 38 [128,1024] Exp ACTIVATEs
x 1.11us on ScalarE were the whole matmul window.  This version:

  - exp runs on TWO engines.  ScalarE keeps exact Exp (scale=ln2,
    bias=-3); ~8 tiles/core instead use a DVE Schraudolph chain:
      u16 = rint(x*1024 + C)          (PSUM f32 -> SBUF u16, ~1.2us)
      u16b = u16 + 512                (int add, 2x mode)
      yb = bitcast_f16(u16b)*2^-0.5   (f16, 2x)
      ex = yb + bitcast_f16(u16)      (f16 add)
    The u16 bit pattern read as f16 is 2^x with the mantissa linearly
    interpolating 2^frac; averaging two half-period phases cuts the
    sawtooth to ~+-0.85% rel.  C folds the f16 exponent bias, the avg2
    pre-halving, the sawtooth centering and the e^-3 exp bias, so
    ScalarE and DVE tiles mix freely within a batch (end-to-end error
    ~1.8e-3).  Only the first op reads PSUM; ops 2-4 are deferred to the
    end of the batch so the score-PSUM recycle never waits on the DVE
    queue's backlog.
  - PE p-state warm-up: the Tensor engine starts at ~1.2GHz and reaches
    2.4GHz only after ~3us of continuous work, and any idle resets it.
    A dummy-matmul burst during the input-DMA dead window plus small
    top-of-slot bursts in the first batches ramp it early and keep it
    ramped; score matmuls then run ~372ns instead of ~630ns.
  - slot order [S3,S4,S5,S7,S6,S2,S1,S0] (ascending rank groups): a
    medium slot first (its exps cover slot 1's DMA), the two largest
    mid-kernel (long continuous PE stretches hold the p-state), the two
    smallest last (short tail).
  - leftover attn@v of a batch is drained AFTER the next batch's score
    emission, never before it, so the burst that unblocks when the last
    exp of a batch lands cannot queue ahead of the scores that feed the
    exp engines.  The drip is sized to drain the WHOLE leftover during
    the next batch's score phase (divisor nxt, not 2*nxt): leaving a
    residual burst at the slot boundary parks 16-48 matmuls in front of
    the following batch's first scores and stalls both exp engines
    ~0.5-1.7us per slot (measured ~2-3us total win from this).
  - epilogue per PSUM half: one [128,4] reciprocal + one broadcast
    (0-stride AP) scalar_tensor_tensor scale for 4 chunks at once, then
    a 128KB half store; the last slot stores in 64KB quarters on both
    DMA queues.
  - pools sized so ex buffers never wrap (no WAR waits; exp waits ride
    on the instructions), inputs prefetched one slot ahead, kT
    dispatched before qT; on the first slot both kT (first 2 key tiles)
    and qT (first 512 cols) are column-split so the first score pair's
    data lands ~1us earlier, and the warm-tile memset runs on the DVE
    so it never delays the gpsimd queue's first input DMA trigger.
"""

import numpy as np

import concourse.bass as bass
import concourse.bacc as bacc
import concourse.tile as tile
from concourse import mybir
from concourse import bass_utils

B, S, D = 64, 1024, 64
NCORES = 8
NB = B // NCORES  # batch slots per core
P = 128
NJT = S // P  # max key tiles per batch
W = D + 1  # v columns + mask column
F32 = mybir.dt.float32
F16 = mybir.dt.float16
U16 = mybir.dt.uint16

LN2 = float(np.log(2.0))
LOG2E = float(np.log2(np.e))
SCH_CORR = 55.0
# u16 = x*1024 + C; C folds the f16 bias (15*1024), the avg2 pre-halving
# (-1024), the sawtooth centering, and the exp bias e^-3 (ScalarE tiles
# use bias=-3, so DVE tiles fold -3*log2e into the exponent domain).
SCH_C = float(15 * 1024 - 1024 - SCH_CORR - 3.0 * LOG2E * 1024)
SQRT_HALF = float(2.0 ** -0.5)

# fraction of each batch's tiles routed to the DVE chain
DVE_FRAC = 0.27

TRACE = False  # set by test harness to capture an NTFF profile
LAST_RESULTS = None  # BassKernelResults stash for the harness

_program_cache = {}


def _av_steps(nc, po_pool, osb_pool, rec_pool, out, s, jt, exs, vm_t,
              last=False):
    """Yield one emission step at a time: 8 attn@v chunk-groups with the
    normalization epilogue emitted per-chunk as each group closes, plus
    half-output DMAs.  The caller interleaves these steps between the NEXT
    batch's score/exp pairs so the PE queue alternates between feeding the
    exp engines (scores) and draining them (attn@v).

    Output accumulators: 8 query-chunks of [128, 65] (cols 0..63 =
    unnormalized out rows, col 64 = denominator); split 4+4 over two PSUM
    banks, groups alternate banks so two can overlap.  As group qc closes,
    its reciprocal+scale run on DVE and the result lands in osb; the
    output store goes out in two 128KB halves (bank A after its last
    chunk, bank B at the end).
    """
    po = [po_pool.tile([P, 4 * W], F32, tag=f"po{h}", name=f"po{h}")
          for h in range(2)]
    order = [0, 4, 1, 5, 2, 6, 3, 7]  # alternate banks
    osb = osb_pool.tile([P, 8 * D], F32, tag="osb", name="osb")
    for qc in order:
        dst = po[qc // 4]
        col = (qc % 4) * W
        for j in range(jt):
            nc.tensor.matmul(
                dst[:, col:col + W],
                lhsT=exs[j][:, qc * P:(qc + 1) * P],
                rhs=vm_t[:, j * W:(j + 1) * W],
                start=(j == 0), stop=(j == jt - 1),
            )
            # fine-grained steps: never queue more than ~4 attn@v matmuls
            # ahead of the next batch's scores, or the exp engines starve
            if j % 4 == 3:
                yield
        if qc in (3, 7):
            # a bank's 4 chunk groups all closed: one [128,4] reciprocal,
            # one broadcast scalar_tensor_tensor scale, half-output DMA
            h = qc // 4
            po3 = po[h].rearrange("p (c w) -> p c w", w=W)
            recp = rec_pool.tile([P, 4], F32, tag=f"rec{h}", name="recp")
            nc.vector.reciprocal(out=recp, in_=po3[:, :, D])
            nc.vector.scalar_tensor_tensor(
                out=osb[:, h * 4 * D:(h + 1) * 4 * D].rearrange(
                    "p (c d) -> p c d", d=D),
                in0=po3[:, :, 0:D], scalar=0.0,
                in1=recp[:, :, None].broadcast_to((P, 4, D)),
                op0=mybir.AluOpType.bypass, op1=mybir.AluOpType.mult)
            if last:
                # tail: quarter stores on both queues so the final
                # transfer is only 64KB
                for qq in range(2):
                    c0 = h * 4 + qq * 2
                    eng = nc.gpsimd if qq == 0 else nc.sync
                    eng.dma_start(
                        out=out[s, c0 * P:(c0 + 2) * P].rearrange(
                            "(c p) d -> p c d", p=P),
                        in_=osb[:, c0 * D:(c0 + 2) * D].rearrange(
                            "p (c d) -> p c d", d=D),
                    )
            else:
                eng = nc.gpsimd if h == 0 else nc.sync
                eng.dma_start(
                    out=out[s, h * 4 * P:(h + 1) * 4 * P].rearrange(
                        "(c p) d -> p c d", p=P),
                    in_=osb[:, h * 4 * D:(h + 1) * 4 * D].rearrange(
                        "p (c d) -> p c d", d=D),
                )
        yield


def _build_program(slots):
    """slots: tuple of (jt, ndve) per batch slot."""
    nc = bacc.Bacc("TRN2", target_bir_lowering=False, debug=False,
                   num_devices=NCORES)
    qT = nc.dram_tensor("qT", [NB, D, S], F16, kind="ExternalInput").ap()
    kT = nc.dram_tensor("kT", [NB, D, S], F16, kind="ExternalInput").ap()
    vm = nc.dram_tensor("vm", [NB, S, W], F16, kind="ExternalInput").ap()
    out = nc.dram_tensor("out", [NB, S, D], F32, kind="ExternalOutput").ap()

    sum_jt = sum(jt for jt, _ in slots)

    with tile.TileContext(nc) as tc:
        with (
            tc.tile_pool(name="singles", bufs=1) as singles,
            tc.tile_pool(name="qk", bufs=3) as qk_pool,
            tc.tile_pool(name="vmp", bufs=4) as vm_pool,
            tc.tile_pool(name="ex", bufs=sum_jt) as ex_pool,
            tc.tile_pool(name="sch", bufs=4) as sch_pool,
            tc.tile_pool(name="osb", bufs=3) as osb_pool,
            tc.tile_pool(name="rec", bufs=2) as rec_pool,
            tc.tile_pool(name="ps_s", bufs=3, space="PSUM") as ps_pool,
            tc.tile_pool(name="ps_o", bufs=1, space="PSUM") as po_pool,
        ):
            # ScalarE tiles: exp(x*ln2 - 3) on x = qk*log2e/8 (qT
            # pre-scaled); -3 bounds the fp16 exp range and cancels
            # between numerator and denominator.
            bias_t = singles.tile([P, 1], F32)
            nc.vector.memset(bias_t, -3.0)

            # PE warm-up: the Tensor engine starts at a low p-state and
            # only reaches full clock after ~3us of continuous work; a
            # burst of dummy matmuls during the otherwise-dead input-DMA
            # window ramps it before the first real score matmul, and
            # small top-of-slot bursts early on keep the ramp alive
            # through the not-yet-pipelined first batches.
            warm = singles.tile([P, 512], F16)
            nc.vector.memset(warm, 1.0)

            def warmup(n):
                wps = ps_pool.tile([P, S], F32, tag="ps", name="wps")
                for _ in range(n):
                    nc.tensor.matmul(wps[:, 0:512], lhsT=warm[:, 0:P],
                                     rhs=warm, start=True, stop=True)

            def emit_input_dmas(s, jt, first=False):
                # q/k replicated into both partition halves (0-stride DMA
                # source) so score matmuls for two key-tiles can run
                # concurrently on PE row-groups (0..63) and (64..127).
                # kT before qT (LDWEIGHTS gates first); on the first slot
                # qT is column-split so the first score pair starts early.
                qT_t = qk_pool.tile([2 * D, S], F16, tag="qT", name="qT_t")
                kT_t = qk_pool.tile([2 * D, S], F16, tag="kT", name="kT_t")
                if first:
                    c0 = min(2 * P, jt * P)
                    nc.sync.dma_start(out=kT_t[0:D, 0:c0],
                                      in_=kT[s, :, 0:c0])
                    nc.gpsimd.dma_start(out=kT_t[D:2 * D, 0:c0],
                                        in_=kT[s, :, 0:c0])
                else:
                    nc.sync.dma_start(out=kT_t[0:D, 0:jt * P],
                                      in_=kT[s, :, 0:jt * P])
                    nc.gpsimd.dma_start(out=kT_t[D:2 * D, 0:jt * P],
                                        in_=kT[s, :, 0:jt * P])
                if first:
                    nc.sync.dma_start(out=qT_t[0:D, 0:512],
                                      in_=qT[s, :, 0:512])
                    nc.gpsimd.dma_start(out=qT_t[D:2 * D, 0:512],
                                        in_=qT[s, :, 0:512])
                    nc.sync.dma_start(out=qT_t[0:D, 512:S],
                                      in_=qT[s, :, 512:S])
                    nc.gpsimd.dma_start(out=qT_t[D:2 * D, 512:S],
                                        in_=qT[s, :, 512:S])
                    if jt * P > c0:
                        nc.sync.dma_start(out=kT_t[0:D, c0:jt * P],
                                          in_=kT[s, :, c0:jt * P])
                        nc.gpsimd.dma_start(out=kT_t[D:2 * D, c0:jt * P],
                                            in_=kT[s, :, c0:jt * P])
                else:
                    nc.sync.dma_start(out=qT_t[0:D, :], in_=qT[s])
                    nc.gpsimd.dma_start(out=qT_t[D:2 * D, :], in_=qT[s])
                # All key tiles of vm in one DMA: [128, jt*65], tile j at
                # columns [j*65, (j+1)*65).
                vm_t = vm_pool.tile([P, NJT * W], F16, tag="vm", name="vm_t")
                nc.sync.dma_start(
                    out=vm_t.rearrange("p (j w) -> p j w", w=W)[:, 0:jt, :],
                    in_=vm[s, 0:jt * P, :].rearrange("(j p) w -> p j w", p=P),
                )
                return qT_t, kT_t, vm_t

            from collections import deque
            pending = deque()  # unfinished attn@v/epilogue generators
            drip = 1
            sch_backlog = []
            warmup(6)
            staged = emit_input_dmas(0, slots[0][0], first=True)
            for s, (jt, ndve) in enumerate(slots):
                qT_t, kT_t, vm_t = staged
                warm_burst = 3 if 1 <= s <= 3 else 0
                wt = (ps_pool.tile([P, S], F32, tag="ps", name="wps")
                      if warm_burst else None)
                if s + 1 < NB:
                    # prefetch the next slot's inputs one slot ahead so
                    # its first score pair never waits on the DMA queue
                    staged = emit_input_dmas(s + 1, slots[s + 1][0])
                # Score matmuls go out in row-group-interleaved pairs --
                # adjacent PE-queue entries on disjoint row groups execute
                # concurrently, so a pair of key tiles costs one tile's time.
                exs = []
                for m in range(0, jt, 2):
                    js = list(range(m, min(m + 2, jt)))
                    pss = [ps_pool.tile([P, S], F32, tag="ps", name="ps")
                           for _ in js]
                    for half in range(2):
                        for r, j in enumerate(js):
                            nc.tensor.matmul(
                                pss[r][:, half * 512:(half + 1) * 512],
                                lhsT=kT_t[r * D:(r + 1) * D,
                                          j * P:(j + 1) * P],
                                rhs=qT_t[r * D:(r + 1) * D,
                                         half * 512:(half + 1) * 512],
                                start=True, stop=True,
                                tile_position=(r * D, 0),
                            )
                    if m == 0 and warm_burst:
                        # p-state hold, placed behind the first pair's
                        # scores instead of in front of them; the tile
                        # was allocated at slot top so the ps ring
                        # rotation matches the original slot-top scheme
                        for _ in range(warm_burst):
                            nc.tensor.matmul(wt[:, 0:512],
                                             lhsT=warm[:, 0:P], rhs=warm,
                                             start=True, stop=True)
                    for r, j in enumerate(js):
                        ex = ex_pool.tile([P, S], F16, tag="ex", name="ex")
                        if j < ndve:
                            # emit only the PSUM-reading op now so the
                            # score buffer frees as fast as an ACTIVATE
                            # would; the SBUF-only ops 2-4 are deferred to
                            # the end of the batch's score phase.
                            ua = sch_pool.tile([P, S], U16, tag="ua",
                                               name="ua")
                            nc.vector.tensor_scalar(
                                out=ua, in0=pss[r], scalar1=1024.0,
                                scalar2=SCH_C, op0=mybir.AluOpType.mult,
                                op1=mybir.AluOpType.add)
                            sch_backlog.append((ua, ex))
                        else:
                            nc.scalar.activation(
                                out=ex, in_=pss[r],
                                func=mybir.ActivationFunctionType.Exp,
                                scale=LN2, bias=bias_t)
                        exs.append(ex)
                        # drain a sliver of the pending attn@v stream
                        # after each exp (keeps the exp engines and PE both
                        # fed), paced to finish just before this batch's
                        # own attn@v
                        for _ in range(drip):
                            if not pending:
                                break
                            if next(pending[0], "done") == "done":
                                pending.popleft()
                for ua, ex in sch_backlog:
                    ub = sch_pool.tile([P, S], U16, tag="ub", name="ub")
                    yb = sch_pool.tile([P, S], F16, tag="yb", name="yb")
                    nc.vector.tensor_scalar(
                        out=ub, in0=ua, scalar1=512, scalar2=None,
                        op0=mybir.AluOpType.add)
                    nc.vector.tensor_scalar(
                        out=yb, in0=ub.bitcast(F16), scalar1=SQRT_HALF,
                        scalar2=None, op0=mybir.AluOpType.mult)
                    nc.vector.tensor_tensor(
                        out=ex, in0=yb, in1=ua.bitcast(F16),
                        op=mybir.AluOpType.add)
                sch_backlog = []
                # drain any leftover attn@v now -- after this batch's
                # scores, so it never blocks them on the PE
                while pending:
                    for _ in pending.popleft():
                        pass
                pending.append(
                    _av_steps(nc, po_pool, osb_pool, rec_pool, out,
                              s, jt, exs, vm_t, last=(s == NB - 1)))
                nsteps = 8 * (jt // 4 + 1)
                nxt = slots[s + 1][0] if s + 1 < NB else jt
                drip = max(1, min(5, -(-nsteps // max(nxt, 1))))
            for gen in pending:
                for _ in gen:
                    pass
    nc.compile()
    return nc


def kernel(q, k, v, valid_lens):
    global LAST_RESULTS
    q = np.array(q, dtype=np.float32, copy=True)
    k = np.asarray(k, dtype=np.float32)
    v = np.asarray(v, dtype=np.float32)
    vl = np.asarray(valid_lens).astype(np.int64)

    # valid_len == 0: reference's softmax over an all-masked row is uniform.
    # Zeroed q gives scores == 0 -> exp == 1 over all (unmasked) keys: same.
    valid_eff = np.where(vl <= 0, S, np.minimum(vl, S))
    q[vl <= 0] = 0.0

    mask = (np.arange(S)[None, :] < valid_eff[:, None]).astype(np.float32)
    # qT carries the 1/8 score scale and log2e: scores become x = s*log2e/8,
    # so exp(s/8) = 2^x for both exp engines.
    qT = np.ascontiguousarray(q.transpose(0, 2, 1) * np.float32(LOG2E / 8))
    qT = qT.astype(np.float16)
    kT = np.ascontiguousarray(k.transpose(0, 2, 1)).astype(np.float16)
    vmh = np.concatenate([v * mask[:, :, None], mask[:, :, None]], axis=2)
    vmh = np.ascontiguousarray(vmh).astype(np.float16)

    # Rank-sort batches; slot s takes one batch of rank group [8s, 8s+8)
    # per core, so the baked per-slot tile count wastes little work.
    # Slot order: ascending, then [.., max, 2nd-max, min]: the pipe fills
    # fast, the big batches' attn@v hides under later exp phases, and the
    # tiny last slot leaves a short tail.
    order = np.argsort(-valid_eff, kind="stable")
    groups = order.reshape(NB, NCORES)[::-1]  # ascending tile counts
    # [S3, S4, S5, S7, S6, S2, S1, S0]: medium slot first (enough exp
    # work to cover the next slot's DMA), the largest batches mid-kernel
    # so the PE gets long continuous stretches (p-state ramp), the two
    # smallest batches last for a short tail.
    perm = [3, 4, 5, NB - 1, NB - 2, 2, 1, 0]
    groups = groups[perm]
    jt_counts = [int(np.ceil(valid_eff[groups[s]].max() / P))
                 for s in range(NB)]
    slots = tuple((jt, int(jt * DVE_FRAC + 0.5) if jt >= 4 else 0)
                  for jt in jt_counts)

    nc = _program_cache.get(slots)
    if nc is None:
        nc = _build_program(slots)
        _program_cache[slots] = nc

    in_maps = []
    for c in range(NCORES):
        bs = groups[:, c]
        in_maps.append({
            "qT": np.ascontiguousarray(qT[bs]),
            "kT": np.ascontiguousarray(kT[bs]),
            "vm": np.ascontiguousarray(vmh[bs]),
        })
    res = bass_utils.run_bass_kernel_spmd(
        nc, in_maps, core_ids=list(range(NCORES)), trace=TRACE,
    )
    LAST_RESULTS = res

    out = np.empty((B, S, D), dtype=np.float32)
    for c in range(NCORES):
        o = res.results[c]["out"]
        for s in range(NB):
            out[groups[s, c]] = o[s]
    return out

